# revision 13
# baseline (speedup 1.0000x reference)
"""DMPNN message-passing kernel for 8 trn2 NeuronCores (SPMD bass/Tile).

 - 4 applications of line-graph operator S (conv_agg_t == S.out_t reused).
 - Dest-edge slices per core; bond edges sorted by dest; scatter via one-hot
   matmuls into per-128-dest-window PSUM.
 - out_t replicated per phase via AllGather; row gathers via indirect DMA.
 - Attention pool via graph-window one-hot matmuls with statically uniform
   (max-over-cores) schedules; deferred gx AllGather; replicated softmax;
   final node scatter via col-sorted windows with ACT-engine sc scaling.
"""
import sys

sys.path.insert(0, "/opt/trn_rl_repo")

import numpy as np
from contextlib import ExitStack

import concourse.bass as bass
import concourse.mybir as mybir
import concourse.tile as tile
from concourse import bacc
from concourse.masks import make_identity
from concourse.bass_utils import run_bass_kernel_spmd

F32 = mybir.dt.float32
I32 = mybir.dt.int32
I16 = mybir.dt.int16
P = 128
NCORES = 8
AF = mybir.ActivationFunctionType
OP = mybir.AluOpType


def _pad_to(x, n, axis=0, val=0):
    pad = [(0, 0)] * x.ndim
    pad[axis] = (0, n - x.shape[axis])
    return np.pad(x, pad, constant_values=val)


def prep(inputs, ncores=NCORES):
    x = np.asarray(inputs["x"], np.float32)
    ea = np.asarray(inputs["edge_attr"], np.float32)
    ei = np.asarray(inputs["edge_index"])
    eib = np.asarray(inputs["edge_index_bond"])
    batch = np.asarray(inputs["edge_index_batch"]).astype(np.int64)
    N, D = x.shape
    E, ED = ea.shape
    B = int(inputs["num_graphs"])
    row, col = ei[0].astype(np.int64), ei[1].astype(np.int64)
    brow, bcol = eib[0].astype(np.int64), eib[1].astype(np.int64)

    EPC = -(-E // (ncores * P)) * P
    EPAD = EPC * ncores
    W = EPC // P
    NPC = -(-N // (ncores * P)) * P
    NPAD = NPC * ncores
    NW = NPC // P
    T = 3
    NLW = 8
    BPAD = P * (-(-B // P))

    meta = dict(N=N, D=D, E=E, ED=ED, B=B, EPC=EPC, EPAD=EPAD, W=W, NPC=NPC,
                NPAD=NPAD, NW=NW, T=T, NLW=NLW, BPAD=BPAD, ncores=ncores)

    # ---- weights ----
    Wu = np.asarray(inputs["W_u"], np.float32) / 3.0
    Wv = np.asarray(inputs["W_v"], np.float32) / 3.0
    We = np.asarray(inputs["W_edge"], np.float32) / 3.0
    WuWv = np.concatenate([Wu, Wv], axis=1)
    wrel = np.asarray(inputs["w_rel"], np.float32).reshape(D, 1)
    wroot = np.asarray(inputs["w_root"], np.float32).reshape(D, 1)
    wrelroot = np.concatenate([wrel, wroot], axis=1)
    Wg = np.asarray(inputs["W_gout"], np.float32)
    b_rel = float(np.asarray(inputs["b_rel"]).reshape(-1)[0])
    brelb = np.full((P, 1), b_rel, np.float32)
    bgout_b = np.broadcast_to(
        np.asarray(inputs["b_gout"], np.float32).reshape(1, D), (P, D)).copy()
    a_mat = np.asarray(inputs["a"], np.float32).reshape(D, T)
    abias_b = np.broadcast_to(
        np.asarray(inputs["a_bias"], np.float32).reshape(1, T), (P, T)).copy()

    # ---- S-phase slots ----
    order = np.argsort(bcol, kind="stable")
    sb_row, sb_col = brow[order], bcol[order]
    GW = EPAD // P
    win_of = sb_col // P
    cnt = np.bincount(win_of, minlength=GW)

    # Per-core window order: sort by bond count (descending) within blocks of
    # 32 positions so the per-position max across cores tightens (fewer
    # gather tiles) while pool graph-locality is preserved.
    BLK = 32
    ord_ = np.zeros((ncores, W), np.int64)
    for k in range(ncores):
        ck = cnt[k * W:(k + 1) * W]
        for b0 in range(0, W, BLK):
            b1 = min(b0 + BLK, W)
            ord_[k, b0:b1] = np.argsort(-ck[b0:b1], kind="stable") + b0
    pos = np.zeros((ncores, W), np.int64)
    for k in range(ncores):
        pos[k, ord_[k]] = np.arange(W)
    # tab-row remap: global (padded) edge index -> row in rebalanced tabs
    eidx = np.arange(EPAD)
    k_e = eidx // EPC
    w_l = (eidx % EPC) // P
    erow = (k_e * EPC + pos[k_e, w_l] * P + eidx % P).astype(np.int64)

    cnt_k = cnt.reshape(ncores, W)
    cnt_pos = np.stack([cnt_k[k][ord_[k]] for k in range(ncores)])
    NTw = np.maximum(1, -(-cnt_pos.max(axis=0) // P))
    NT_S = int(NTw.sum())
    tstart = np.concatenate([[0], np.cumsum(NTw)]).astype(int)
    wstart = np.concatenate([[0], np.cumsum(cnt)]).astype(int)
    sidx = np.zeros((ncores, NT_S * P), np.int32)
    sdloc = np.full((ncores, NT_S * P), 255, np.int16)
    for k in range(ncores):
        for j in range(W):
            g = k * W + int(ord_[k, j])
            n = wstart[g + 1] - wstart[g]
            o = tstart[j] * P
            sidx[k, o:o + n] = erow[sb_row[wstart[g]:wstart[g + 1]]]
            sdloc[k, o:o + n] = (sb_col[wstart[g]:wstart[g + 1]] % P)
    sidx = sidx.reshape(ncores, NT_S, P).transpose(0, 2, 1).copy()
    sdloc = sdloc.reshape(ncores, NT_S, P).transpose(0, 2, 1).copy()

    # ---- base gather indices (columns permuted per core) ----
    ridx = _pad_to(row, EPAD).reshape(ncores, W, P)
    cidx = _pad_to(col, EPAD).reshape(ncores, W, P)
    ridx = np.stack([ridx[k][ord_[k]] for k in range(ncores)])
    cidx = np.stack([cidx[k][ord_[k]] for k in range(ncores)])
    ridx = ridx.transpose(0, 2, 1).astype(np.int32).copy()
    cidx = cidx.transpose(0, 2, 1).astype(np.int32).copy()

    # ---- pool: static union tile-ranges per local graph-window ----
    batch_p = _pad_to(batch, EPAD, val=B)
    bpc = batch_p.reshape(ncores, W, P)
    bpc = np.stack([bpc[k][ord_[k]] for k in range(ncores)])
    LG0 = np.zeros(ncores, np.int64)
    for k in range(ncores):
        real = bpc[k][bpc[k] < B]
        LG0[k] = 128 * ((real.min() // 128) if real.size else 0)
    Sj = np.full(NLW, W, np.int64)
    Ej = np.zeros(NLW, np.int64)
    for k in range(ncores):
        for j in range(NLW):
            lo, hi = LG0[k] + 128 * j, LG0[k] + 128 * (j + 1)
            m = (bpc[k] >= lo) & (bpc[k] < hi)
            tm = m.any(axis=1)
            if tm.any():
                tt = np.nonzero(tm)[0]
                Sj[j] = min(Sj[j], tt[0])
                Ej[j] = max(Ej[j], tt[-1] + 1)
    Sj = np.minimum(Sj, Ej)
    PTOT = int((Ej - Sj).sum())
    pstart = np.concatenate([[0], np.cumsum(Ej - Sj)]).astype(int)
    ppool = np.full((ncores, P, PTOT), 255, np.int16)
    for k in range(ncores):
        for j in range(NLW):
            for ti, t in enumerate(range(Sj[j], Ej[j])):
                rel = bpc[k, t] - (LG0[k] + 128 * j)
                v = np.where((rel >= 0) & (rel < 128), rel, 255)
                ppool[k, :, pstart[j] + ti] = v.astype(np.int16)
    meta.update(LG0=LG0, Sj=Sj, Ej=Ej, pstart=pstart, PTOT=PTOT)

    # ---- final node-window slots ----
    orderf = np.argsort(col, kind="stable")
    f_e, f_col = orderf, col[orderf]
    GNW = NPAD // P
    fcnt = np.bincount(f_col // P, minlength=GNW)
    NTf = np.maximum(1, -(-fcnt.reshape(ncores, NW).max(axis=0) // P))
    NT_F = int(NTf.sum())
    ftstart = np.concatenate([[0], np.cumsum(NTf)]).astype(int)
    fwstart = np.concatenate([[0], np.cumsum(fcnt)]).astype(int)
    fidx = np.zeros((ncores, NT_F * P), np.int32)
    fdloc = np.full((ncores, NT_F * P), 255, np.int16)
    for k in range(ncores):
        for w in range(NW):
            g = k * NW + w
            n = fwstart[g + 1] - fwstart[g]
            o = ftstart[w] * P
            sl = slice(fwstart[g], fwstart[g + 1])
            fidx[k, o:o + n] = erow[f_e[sl]]
            fdloc[k, o:o + n] = (f_col[sl] % P)
    fidx = fidx.reshape(ncores, NT_F, P).transpose(0, 2, 1).copy()
    fdloc = fdloc.reshape(ncores, NT_F, P).transpose(0, 2, 1).copy()

    # ---- per-edge batch ids (for sce gather in out_final build) ----
    ebat = _pad_to(batch, EPAD, val=0).astype(np.int32).reshape(ncores, W, P)
    ebat = np.stack([ebat[k][ord_[k]] for k in range(ncores)])
    ebat = ebat.transpose(0, 2, 1).copy()

    meta.update(NTw=NTw, tstart=tstart, NT_S=NT_S, NTf=NTf, ftstart=ftstart,
                NT_F=NT_F)

    xpad = _pad_to(x, NPAD)
    xT = xpad.T.copy()
    eaT = _pad_to(ea, EPAD).T.copy()   # [ED, EPAD]

    in_maps = []
    for k in range(ncores):
        in_maps.append({
            "xT": np.ascontiguousarray(xT[:, k * NPC:(k + 1) * NPC]),
            "xw": np.ascontiguousarray(xpad[k * NPC:(k + 1) * NPC]),
            "eaT": np.ascontiguousarray(
                eaT[:, k * EPC:(k + 1) * EPC].reshape(ED, W, P)[:, ord_[k], :]
                .reshape(ED, EPC)),
            "WuWv": WuWv, "We": We, "wrelroot": wrelroot, "Wg": Wg,
            "brelb": brelb, "bgoutb": bgout_b, "amat": a_mat,
            "abiasb": abias_b,
            "sidx": sidx[k], "sdloc": sdloc[k],
            "ridx": ridx[k], "cidx": cidx[k], "ppool": ppool[k],
            "fidx": fidx[k], "fdloc": fdloc[k], "ebat": ebat[k],
        })
    return in_maps, meta


def build_program(meta):
    D, ED = meta["D"], meta["ED"]
    EPC, EPAD, W = meta["EPC"], meta["EPAD"], meta["W"]
    NPC, NPAD, NW = meta["NPC"], meta["NPAD"], meta["NW"]
    T, NLW, BPAD = meta["T"], meta["NLW"], meta["BPAD"]
    NTw, tstart, NT_S = meta["NTw"], meta["tstart"], meta["NT_S"]
    NTf, ftstart, NT_F = meta["NTf"], meta["ftstart"], meta["NT_F"]
    Sj, Ej, pstart, PTOT = meta["Sj"], meta["Ej"], meta["pstart"], meta["PTOT"]
    LG0 = meta["LG0"]
    ncores = meta["ncores"]
    GL = 4
    DP = D + 4

    SPANS = max(int(NTw[t0:min(t0 + GL, W)].sum()) for t0 in range(0, W, GL))
    SPANF = max(int(NTf[t0:min(t0 + GL, NW)].sum()) for t0 in range(0, NW, GL))

    nc = bacc.Bacc("TRN2", target_bir_lowering=False, debug=False,
                   num_devices=ncores)

    def param(name, shape, dt):
        return nc.declare_dram_parameter(name, shape, dt, isOutput=False)

    pxT = param("xT", [D, NPC], F32)
    pxw = param("xw", [NPC, D], F32)
    peaT = param("eaT", [ED, EPC], F32)
    pWuWv = param("WuWv", [D, 2 * D], F32)
    pWe = param("We", [ED, D], F32)
    pwrr = param("wrelroot", [D, 2], F32)
    pWg = param("Wg", [D, D], F32)
    pbrelb = param("brelb", [P, 1], F32)
    pbgoutb = param("bgoutb", [P, D], F32)
    pamat = param("amat", [D, T], F32)
    pabiasb = param("abiasb", [P, T], F32)
    psidx = param("sidx", [P, NT_S], I32)
    psdloc = param("sdloc", [P, NT_S], I16)
    pridx = param("ridx", [P, W], I32)
    pcidx = param("cidx", [P, W], I32)
    pppool = param("ppool", [P, PTOT], I16)
    pfidx = param("fidx", [P, NT_F], I32)
    pfdloc = param("fdloc", [P, NT_F], I16)
    pebat = param("ebat", [P, W], I32)
    pout = nc.declare_dram_parameter("out", [NPC, D], F32, isOutput=True)
    import os
    DBG = bool(int(os.environ.get("KERNEL_DEBUG", "0")))
    STAGE = int(os.environ.get("KERNEL_STAGE", "9"))
    GTB = int(os.environ.get("KERNEL_GTBUFS", "16"))
    if DBG:
        dbg_base = nc.declare_dram_parameter("dbg_base", [EPC, D], F32,
                                             isOutput=True)
        dbg_o = [nc.declare_dram_parameter(f"dbg_o{t}", [EPC, D], F32,
                                           isOutput=True) for t in range(T)]
        dbg_xc = nc.declare_dram_parameter("dbg_xc", [P, T * W], F32,
                                           isOutput=True)
        dbg_sc = nc.declare_dram_parameter("dbg_sc", [BPAD, 64], F32,
                                           isOutput=True)
        dbg_gxl = nc.declare_dram_parameter("dbg_gxl", [T * NLW * P, DP], F32,
                                            isOutput=True)

    xauv_slice = nc.dram_tensor("xauv_slice", [NPC, 2 * D], F32)
    xauv_tab = nc.dram_tensor("xauv_tab", [NPAD, 2 * D], F32, addr_space="Shared")
    base_slice = nc.dram_tensor("base_slice", [EPC, D], F32)
    base_tab = nc.dram_tensor("base_tab", [EPAD, D], F32, addr_space="Shared")
    out_slice = [nc.dram_tensor(f"out_slice{t}", [EPC, D], F32) for t in range(T)]
    out_tab = [nc.dram_tensor(f"out_tab{t}", [EPAD, D], F32, addr_space="Shared")
               for t in range(T - 1)]
    d3_slice = nc.dram_tensor("d3_slice", [EPC, 2], F32)
    d3_tab = nc.dram_tensor("d3_tab", [EPAD, 2], F32, addr_space="Shared")
    of_slice = nc.dram_tensor("of_slice", [EPC, D], F32)
    of_tab = nc.dram_tensor("of_tab", [EPAD, D], F32, addr_space="Shared")
    gxl = nc.dram_tensor("gxl", [T * NLW * P, DP], F32)
    gx_all = nc.dram_tensor("gx_all", [ncores * T * NLW * P, DP], F32,
                            addr_space="Shared")
    sc_tab = nc.dram_tensor("sc_tab", [BPAD, 64], F32)

    rg = [list(range(ncores))]

    with tile.TileContext(nc) as tc, ExitStack() as ctx:
        sb = ctx.enter_context(tc.tile_pool(name="sb", bufs=2))
        sbc = ctx.enter_context(tc.tile_pool(name="sbc", bufs=1))
        ps = ctx.enter_context(tc.tile_pool(name="ps", bufs=4, space="PSUM"))
        ps2 = ctx.enter_context(tc.tile_pool(name="ps2", bufs=2, space="PSUM"))
        psg = ctx.enter_context(tc.tile_pool(name="psg", bufs=2, space="PSUM"))

        def cload(name, pp, shape, dt):
            t = sbc.tile(shape, dt, tag=name)
            nc.sync.dma_start(out=t[:], in_=pp[:])
            return t

        c_WuWv = cload("WuWv", pWuWv, [D, 2 * D], F32)
        c_We = cload("We", pWe, [ED, D], F32)
        c_wrr = cload("wrr", pwrr, [D, 2], F32)
        c_Wg = cload("Wg", pWg, [D, D], F32)
        c_brelb = cload("brelb", pbrelb, [P, 1], F32)
        c_bgoutb = cload("bgoutb", pbgoutb, [P, D], F32)
        c_amat = cload("amat", pamat, [D, T], F32)
        c_abiasb = cload("abiasb", pabiasb, [P, T], F32)
        c_sidx = cload("sidx", psidx, [P, NT_S], I32)
        c_sdloc = cload("sdloc", psdloc, [P, NT_S], I16)
        c_ridx = cload("ridx", pridx, [P, W], I32)
        c_cidx = cload("cidx", pcidx, [P, W], I32)
        c_ppool = cload("ppool", pppool, [P, PTOT], I16)
        c_fidx = cload("fidx", pfidx, [P, NT_F], I32)
        c_fdloc = cload("fdloc", pfdloc, [P, NT_F], I16)
        c_ebat = cload("ebat", pebat, [P, W], I32)

        c_iota = sbc.tile([P, P], I16, tag="iota")
        nc.gpsimd.iota(c_iota[:], pattern=[[1, P]], base=0, channel_multiplier=0)
        c_ident = sbc.tile([P, P], F32, tag="ident")
        make_identity(nc, c_ident[:])

        def eq_mask(out_t, loc_ap, n):
            nc.vector.tensor_tensor(
                out=out_t[:, :n * P].rearrange("p (j q) -> p j q", j=n),
                in0=loc_ap[:, :, None].to_broadcast([P, n, P]),
                in1=c_iota[:, None, :].to_broadcast([P, n, P]),
                op=OP.is_equal)

        def grp_store(dram, t0, g, stg, width):
            nc.sync.dma_start(
                out=dram[t0 * P:(t0 + g) * P, :].rearrange("(a p) d -> p a d", p=P),
                in_=stg[:, :g * width].rearrange("p (a d) -> p a d", a=g))

        def grp_load(stg, dram, t0, g, width):
            nc.sync.dma_start(
                out=stg[:, :g * width].rearrange("p (a d) -> p a d", a=g),
                in_=dram[t0 * P:(t0 + g) * P, :].rearrange("(a p) d -> p a d", p=P))

        # ============ stage A: xau|xav slices ============
        for t0 in range(0, NW, GL):
            g = min(GL, NW - t0)
            xtl = sb.tile([P, GL * P], F32, tag="xtl")
            nc.sync.dma_start(out=xtl[:, :g * P],
                              in_=pxT[:, t0 * P:(t0 + g) * P])
            stg = sb.tile([P, GL * 2 * D], F32, tag="stgA")
            for j in range(g):
                t = t0 + j
                pa = ps.tile([P, 2 * D], F32, tag="ps1")
                nc.tensor.matmul(pa[:], xtl[:, j * P:(j + 1) * P], c_WuWv[:],
                                 start=True, stop=True)
                nc.vector.tensor_copy(out=stg[:, j * 2 * D:(j + 1) * 2 * D],
                                      in_=pa[:])
            grp_store(xauv_slice, t0, g, stg, 2 * D)
        nc.gpsimd.collective_compute("AllGather", OP.bypass, replica_groups=rg,
                                     ins=[xauv_slice[:]], outs=[xauv_tab[:]])

        def bail():
            for t0 in range(0, NW, GL):
                g = min(GL, NW - t0)
                xl = sb.tile([P, GL * D], F32, tag="xl")
                grp_load(xl, pxw, t0, g, D)
                grp_store(pout, t0, g, xl, D)

        # ============ stage B: base ============
        for t0 in range(0, W, GL) if STAGE >= 1 else []:
            g = min(GL, W - t0)
            eal = sb.tile([ED, GL * P], F32, tag="eal")
            nc.sync.dma_start(out=eal[:, :g * P],
                              in_=peaT[:, t0 * P:(t0 + g) * P])
            stg = sb.tile([P, GL * D], F32, tag="stgB")
            for j in range(g):
                t = t0 + j
                g1 = sb.tile([P, D], F32, tag="g1", bufs=6)
                nc.gpsimd.indirect_dma_start(
                    out=g1[:], out_offset=None, in_=xauv_tab[:],
                    in_offset=bass.IndirectOffsetOnAxis(ap=c_ridx[:, t:t + 1],
                                                        axis=0))
                g2 = sb.tile([P, D], F32, tag="g2", bufs=6)
                nc.gpsimd.indirect_dma_start(
                    out=g2[:], out_offset=None, in_=xauv_tab[:],
                    in_offset=bass.IndirectOffsetOnAxis(ap=c_cidx[:, t:t + 1],
                                                        axis=0),
                    element_offset=D)
                pe = ps.tile([P, 2 * D], F32, tag="ps1")
                nc.tensor.matmul(pe[:, :D], eal[:, j * P:(j + 1) * P],
                                 c_We[:], start=True, stop=True)
                s = sb.tile([P, D], F32, tag="bsum")
                nc.vector.tensor_add(out=s[:], in0=g1[:], in1=g2[:])
                nc.vector.tensor_add(out=stg[:, j * D:(j + 1) * D], in0=s[:],
                                     in1=pe[:, :D])
            grp_store(base_slice, t0, g, stg, D)
        if STAGE >= 1:
            nc.gpsimd.collective_compute("AllGather", OP.bypass, replica_groups=rg,
                                         ins=[base_slice[:]], outs=[base_tab[:]])

        # bv' = base@wrel - b_rel per window (for xc assembly)
        bv = sbc.tile([P, W], F32, tag="bv")
        for t0 in range(0, W, GL) if STAGE >= 1 else []:
            g = min(GL, W - t0)
            bl = sb.tile([P, GL * D], F32, tag="bload")
            grp_load(bl, base_slice, t0, g, D)
            for j in range(g):
                t = t0 + j
                pt = ps.tile([P, 2 * D], F32, tag="ps1")
                nc.tensor.transpose(out=pt[:, :P], in_=bl[:, j * D:(j + 1) * D],
                                    identity=c_ident[:])
                ts = sb.tile([P, P], F32, tag="tsb", bufs=4)
                nc.vector.tensor_copy(out=ts[:], in_=pt[:, :P])
                pv = ps2.tile([P, 2], F32, tag="ps2")
                nc.tensor.matmul(pv[:], ts[:], c_wrr[:], start=True, stop=True)
                nc.vector.tensor_sub(out=bv[:, t:t + 1], in0=pv[:, 0:1],
                                     in1=c_brelb[:])

        # ============ phases 1..4 ============
        xc_cols = [None] + [sbc.tile([P, W], F32, tag=f"xc{t}", name=f"xc{t}")
                            for t in range(1, T + 1)]
        or_cols = [None] + [sbc.tile([P, W], F32, tag=f"or{t}", name=f"or{t}")
                            for t in range(1, T + 1)]
        tabs = [base_tab, out_tab[0], out_tab[1], d3_tab]
        for ph in range(1, min(5, 2 + max(0, STAGE - 2))):
            tab = tabs[ph - 1]
            for t0 in range(0, W, GL):
                g = min(GL, W - t0)
                nt_tot = int(NTw[t0:t0 + g].sum())
                mfirst = int(tstart[t0])
                mm = sb.tile([P, SPANS * P], F32, tag="mm")
                eq_mask(mm, c_sdloc[:, mfirst:mfirst + nt_tot], nt_tot)
                if ph <= 3:
                    bl = sb.tile([P, GL * D], F32, tag="phb")
                    grp_load(bl, base_slice, t0, g, D)
                    stg = sb.tile([P, GL * D], F32, tag="phs")
                if ph == 3:
                    stg2 = sb.tile([P, GL * 2], F32, tag="d3stg")
                for j in range(g):
                    w = t0 + j
                    nt = int(NTw[w])
                    if ph == 4:
                        # scalar-only phase: gather [d3|s3] pairs, accumulate
                        # S(out_3)@wrel per dest window in a [P,2] psum
                        pc2 = ps2.tile([P, 2], F32, tag="ps2")
                        for i in range(nt):
                            tt = int(tstart[w]) + i
                            gt4 = sb.tile([P, 2], F32, tag="gt4", bufs=GTB)
                            nc.gpsimd.indirect_dma_start(
                                out=gt4[:], out_offset=None, in_=tab[:],
                                in_offset=bass.IndirectOffsetOnAxis(
                                    ap=c_sidx[:, tt:tt + 1], axis=0))
                            mo = (tt - mfirst) * P
                            nc.tensor.matmul(pc2[:], mm[:, mo:mo + P], gt4[:],
                                             start=(i == 0), stop=(i == nt - 1))
                        # xc_3 = S(out_3)@wrel + b_rel + or_3
                        tmp = sb.tile([P, 1], F32, tag="xctmp")
                        nc.vector.tensor_add(out=tmp[:], in0=pc2[:, 0:1],
                                             in1=c_brelb[:])
                        nc.vector.tensor_add(out=xc_cols[3][:, w:w + 1],
                                             in0=tmp[:],
                                             in1=or_cols[3][:, w:w + 1])
                        continue
                    pc = ps.tile([P, 2 * D], F32, tag="ps1")
                    for i in range(nt):
                        tt = int(tstart[w]) + i
                        gt = sb.tile([P, D], F32, tag="gt", bufs=GTB)
                        nc.gpsimd.indirect_dma_start(
                            out=gt[:], out_offset=None, in_=tab[:],
                            in_offset=bass.IndirectOffsetOnAxis(
                                ap=c_sidx[:, tt:tt + 1], axis=0))
                        mo = (tt - mfirst) * P
                        nc.tensor.matmul(pc[:, :D], mm[:, mo:mo + P], gt[:],
                                         start=(i == 0), stop=(i == nt - 1))
                    ow = stg[:, j * D:(j + 1) * D]
                    nc.vector.tensor_add(out=ow, in0=pc[:, :D],
                                         in1=bl[:, j * D:(j + 1) * D])
                    src = ow
                    pt = ps.tile([P, 2 * D], F32, tag="ps1")
                    nc.tensor.transpose(out=pt[:, :P], in_=src,
                                        identity=c_ident[:])
                    ts = sb.tile([P, P], F32, tag="tsb", bufs=4)
                    nc.vector.tensor_copy(out=ts[:], in_=pt[:, :P])
                    pv = ps2.tile([P, 2], F32, tag="ps2")
                    nc.tensor.matmul(pv[:], ts[:], c_wrr[:], start=True,
                                     stop=True)
                    nc.vector.tensor_copy(out=or_cols[ph][:, w:w + 1],
                                          in_=pv[:, 1:2])
                    if ph == 3:
                        nc.vector.tensor_copy(
                            out=stg2[:, j * 2:(j + 1) * 2], in_=pv[:])
                    if ph >= 2:
                        # xc_{ph-1} = ov_ph - bv' + or_{ph-1}
                        tmp = sb.tile([P, 1], F32, tag="xctmp")
                        nc.vector.tensor_sub(out=tmp[:], in0=pv[:, 0:1],
                                             in1=bv[:, w:w + 1])
                        nc.vector.tensor_add(
                            out=xc_cols[ph - 1][:, w:w + 1], in0=tmp[:],
                            in1=or_cols[ph - 1][:, w:w + 1])
                if ph <= 3:
                    grp_store(out_slice[ph - 1], t0, g, stg, D)
                if ph == 3:
                    grp_store(d3_slice, t0, g, stg2, 2)
            if ph <= 2:
                nc.gpsimd.collective_compute(
                    "AllGather", OP.bypass, replica_groups=rg,
                    ins=[out_slice[ph - 1][:]], outs=[out_tab[ph - 1][:]])
            if ph == 3:
                nc.gpsimd.collective_compute(
                    "AllGather", OP.bypass, replica_groups=rg,
                    ins=[d3_slice[:]], outs=[d3_tab[:]])

        # ============ pool (iterations 1..3) ============
        for it in range(1, (T + 1) if STAGE >= 6 else 1):
            gxs = sb.tile([P, NLW * DP], F32, tag="gxs")
            nc.gpsimd.memset(gxs[:], 0.0)
            for j in range(NLW):
                s0, e0 = int(Sj[j]), int(Ej[j])
                if e0 <= s0:
                    continue
                pg = psg.tile([P, DP], F32, tag="psg")
                first = True
                for t0 in range(s0, e0, GL):
                    g = min(GL, e0 - t0)
                    ol = sb.tile([P, GL * D], F32, tag="plod")
                    grp_load(ol, out_slice[it - 1], t0, g, D)
                    mA = sb.tile([P, GL * P], F32, tag="mA")
                    po = int(pstart[j]) + (t0 - s0)
                    eq_mask(mA, c_ppool[:, po:po + g], g)
                    for jj in range(g):
                        t = t0 + jj
                        ex = sb.tile([P, 1], F32, tag="ex")
                        nc.scalar.activation(out=ex[:],
                                             in_=xc_cols[it][:, t:t + 1],
                                             func=AF.Exp)
                        rhs = sb.tile([P, D + 1], F32, tag="prhs")
                        nc.scalar.activation(out=rhs[:, :D],
                                             in_=ol[:, jj * D:(jj + 1) * D],
                                             func=AF.Copy, scale=ex[:])
                        nc.vector.tensor_copy(out=rhs[:, D:D + 1], in_=ex[:])
                        nc.tensor.matmul(pg[:, :D + 1],
                                         mA[:, jj * P:(jj + 1) * P], rhs[:],
                                         start=first, stop=(t == e0 - 1))
                        first = False
                nc.vector.tensor_copy(out=gxs[:, j * DP:j * DP + D + 1],
                                      in_=pg[:, :D + 1])
            nc.sync.dma_start(
                out=gxl[(it - 1) * NLW * P:it * NLW * P, :].rearrange(
                    "(a p) d -> p a d", p=P),
                in_=gxs[:].rearrange("p (a d) -> p a d", a=NLW))
        if STAGE >= 6:
            nc.gpsimd.collective_compute("AllGather", OP.bypass, replica_groups=rg,
                                         ins=[gxl[:]], outs=[gx_all[:]])

        # ============ sc (replicated) ============
        SC_ON = STAGE >= 7
        contrib = {}
        for k in range(ncores):
            for j in range(NLW):
                gw = int((LG0[k] + 128 * j) // 128)
                if gw * P < BPAD:
                    contrib.setdefault(gw, []).append((k, j))
        for gw in range(BPAD // P) if SC_ON else []:
            zz = sb.tile([P, T], F32, tag="zz")
            for it in range(1, T + 1):
                gxg = sb.tile([P, DP], F32, tag="gxg")
                srcs = contrib.get(gw, [])
                if not srcs:
                    nc.gpsimd.memset(gxg[:], 0.0)
                else:
                    for si, (k, j) in enumerate(srcs):
                        roff = (k * T * NLW + (it - 1) * NLW + j) * P
                        if si == 0:
                            nc.sync.dma_start(out=gxg[:],
                                              in_=gx_all[roff:roff + P, :])
                        else:
                            tmp2 = sb.tile([P, DP], F32, tag="gxg2")
                            nc.sync.dma_start(out=tmp2[:],
                                              in_=gx_all[roff:roff + P, :])
                            nc.vector.tensor_add(out=gxg[:], in0=gxg[:],
                                                 in1=tmp2[:])
                den = sb.tile([P, 1], F32, tag="den")
                nc.vector.tensor_scalar_add(out=den[:], in0=gxg[:, D:D + 1],
                                            scalar1=1e-16)
                rd = sb.tile([P, 1], F32, tag="rd")
                nc.vector.reciprocal(out=rd[:], in_=den[:])
                gxn = sb.tile([P, D], F32, tag="gxn")
                nc.scalar.activation(out=gxn[:], in_=gxg[:, :D], func=AF.Copy,
                                     scale=rd[:])
                ptr = ps.tile([P, 2 * D], F32, tag="ps1")
                nc.tensor.transpose(out=ptr[:, :P], in_=gxn[:],
                                    identity=c_ident[:])
                gxnT = sb.tile([P, D], F32, tag="gxnT")
                nc.vector.tensor_copy(out=gxnT[:], in_=ptr[:, :P])
                pgo = ps.tile([P, 2 * D], F32, tag="ps1")
                nc.tensor.matmul(pgo[:, :D], gxnT[:], c_Wg[:], start=True,
                                 stop=True)
                gsum = sb.tile([P, D], F32, tag="gsum")
                nc.vector.tensor_add(out=gsum[:], in0=pgo[:, :D],
                                     in1=c_bgoutb[:])
                gout = sb.tile([P, D], F32, tag="gout")
                nc.scalar.activation(out=gout[:], in_=gsum[:], func=AF.Tanh)
                ptr2 = ps.tile([P, 2 * D], F32, tag="ps1")
                nc.tensor.transpose(out=ptr2[:, :P], in_=gout[:],
                                    identity=c_ident[:])
                goutT = sb.tile([P, D], F32, tag="goutT")
                nc.vector.tensor_copy(out=goutT[:], in_=ptr2[:, :P])
                pz = ps2.tile([P, 2], F32, tag="ps2")
                nc.tensor.matmul(pz[:, 0:1], goutT[:],
                                 c_amat[:, it - 1:it], start=True, stop=True)
                nc.vector.tensor_copy(out=zz[:, it - 1:it], in_=pz[:, 0:1])
            z2 = sb.tile([P, T], F32, tag="z2")
            nc.vector.tensor_add(out=z2[:], in0=zz[:], in1=c_abiasb[:])
            nm = sb.tile([P, 1], F32, tag="nm")
            nc.vector.tensor_reduce(out=nm[:], in_=z2[:],
                                    axis=mybir.AxisListType.X, op=OP.max,
                                    negate=True)
            esc = sb.tile([P, T], F32, tag="esc")
            se = sb.tile([P, 1], F32, tag="se")
            nc.scalar.activation(out=esc[:], in_=z2[:], func=AF.Exp,
                                 bias=nm[:], accum_out=se[:])
            rse = sb.tile([P, 1], F32, tag="rse")
            nc.vector.reciprocal(out=rse[:], in_=se[:])
            scs = sb.tile([P, T], F32, tag="scs")
            nc.scalar.activation(out=scs[:], in_=esc[:], func=AF.Copy,
                                 scale=rse[:])
            nc.sync.dma_start(out=sc_tab[gw * P:(gw + 1) * P, 0:T], in_=scs[:])

        # ============ out_final build (edge-local) ============
        for t0 in range(0, W, GL) if STAGE >= 8 else []:
            g = min(GL, W - t0)
            ol0 = sb.tile([P, GL * D], F32, tag="ofl0")
            ol1 = sb.tile([P, GL * D], F32, tag="ofl1")
            ol2 = sb.tile([P, GL * D], F32, tag="ofl2")
            grp_load(ol0, out_slice[0], t0, g, D)
            grp_load(ol1, out_slice[1], t0, g, D)
            grp_load(ol2, out_slice[2], t0, g, D)
            stg = sb.tile([P, GL * D], F32, tag="ofstg")
            for j in range(g):
                w = t0 + j
                sce = sb.tile([P, 64], F32, tag="ofsce", bufs=4)
                nc.gpsimd.indirect_dma_start(
                    out=sce[:], out_offset=None, in_=sc_tab[:],
                    in_offset=bass.IndirectOffsetOnAxis(
                        ap=c_ebat[:, w:w + 1], axis=0))
                a1 = sb.tile([P, D], F32, tag="ofa1", bufs=4)
                a2 = sb.tile([P, D], F32, tag="ofa2", bufs=4)
                a3 = sb.tile([P, D], F32, tag="ofa3", bufs=4)
                nc.scalar.activation(out=a1[:], in_=ol0[:, j * D:(j + 1) * D],
                                     func=AF.Copy, scale=sce[:, 0:1])
                nc.scalar.activation(out=a2[:], in_=ol1[:, j * D:(j + 1) * D],
                                     func=AF.Copy, scale=sce[:, 1:2])
                nc.scalar.activation(out=a3[:], in_=ol2[:, j * D:(j + 1) * D],
                                     func=AF.Copy, scale=sce[:, 2:3])
                a12 = sb.tile([P, D], F32, tag="ofa12", bufs=4)
                nc.vector.tensor_add(out=a12[:], in0=a1[:], in1=a2[:])
                nc.vector.tensor_add(out=stg[:, j * D:(j + 1) * D],
                                     in0=a12[:], in1=a3[:])
            grp_store(of_slice, t0, g, stg, D)
        if STAGE >= 8:
            nc.gpsimd.collective_compute("AllGather", OP.bypass,
                                         replica_groups=rg,
                                         ins=[of_slice[:]], outs=[of_tab[:]])

        # ============ final: node windows ============
        if STAGE < 8:
            bail()
        for t0 in range(0, NW, GL) if STAGE >= 8 else []:
            g = min(GL, NW - t0)
            sp0, sp1 = int(ftstart[t0]), int(ftstart[t0 + g])
            span = sp1 - sp0
            mm = sb.tile([P, SPANF * P], F32, tag="fmm")
            eq_mask(mm, c_fdloc[:, sp0:sp1], span)

            xl = sb.tile([P, GL * D], F32, tag="xl")
            grp_load(xl, pxw, t0, g, D)
            stg = sb.tile([P, GL * D], F32, tag="fstg")
            for j in range(g):
                w = t0 + j
                nt = int(NTf[w])
                pf = ps.tile([P, 2 * D], F32, tag="ps1")
                for i in range(nt):
                    tt = int(ftstart[w]) + i
                    rel = tt - sp0
                    gt = sb.tile([P, D], F32, tag="fgt", bufs=GTB)
                    nc.gpsimd.indirect_dma_start(
                        out=gt[:], out_offset=None, in_=of_tab[:],
                        in_offset=bass.IndirectOffsetOnAxis(
                            ap=c_fidx[:, tt:tt + 1], axis=0))
                    nc.tensor.matmul(pf[:, :D],
                                     mm[:, rel * P:(rel + 1) * P], gt[:],
                                     start=(i == 0), stop=(i == nt - 1))
                nc.vector.tensor_add(out=stg[:, j * D:(j + 1) * D],
                                     in0=pf[:, :D], in1=xl[:, j * D:(j + 1) * D])
            grp_store(pout, t0, g, stg, D)

        if DBG:
            nc.gpsimd.dma_start(out=dbg_base[:], in_=base_slice[:])
            for t in range(T):
                nc.gpsimd.dma_start(out=dbg_o[t][:], in_=out_slice[t][:])
            for t in range(1, T + 1):
                nc.sync.dma_start(out=dbg_xc[:, (t - 1) * W:t * W],
                                  in_=xc_cols[t][:])
            nc.gpsimd.dma_start(out=dbg_sc[:], in_=sc_tab[:])
            nc.gpsimd.dma_start(out=dbg_gxl[:], in_=gxl[:])

    nc.finalize()
    return nc


def _run(in_maps, meta):
    nc = build_program(meta)
    r = run_bass_kernel_spmd(nc, in_maps, list(range(meta["ncores"])),
                             trace=False)
    return r


def kernel(**inputs):
    in_maps, meta = prep(inputs)
    r = _run(in_maps, meta)
    N, NPC, D = meta["N"], meta["NPC"], meta["D"]
    out = np.concatenate([r.results[k]["out"] for k in range(meta["ncores"])],
                         axis=0)[:N]
    return out.astype(np.float32)

